# revision 13
# baseline (speedup 1.0000x reference)
"""Trainium2 Bass kernel for nn_AttentionBranch: conv->relu->maxpool->conv->relu
followed by per-location rank-1 Gram outer products (100, 1024, 1024).

Sharding: the 100-location Gram axis is split across 8 NeuronCores
(13/12 locations per core). The conv backbone is replicated (conv1) /
channel-sliced to each core's needed 136-channel window (conv2), so no
collectives are required. The row-major .view(100, 1024) of the conv2
output is realised through a tiny DRAM scratch roundtrip.

Numerics: conv1/conv2 matmuls run in fp16 (single-pass on TensorE, fp32
PSUM accumulation); gram-phase broadcasts/transposes use f32r (hi/lo
split, ~1e-5); the Gram products are fp32 on VectorE/ScalarE. Total rel
err ~7e-4 vs the 2e-2 gate.

The per-core flat-offset select (delta 0 or 12) is folded into the
contraction dim of the gram matmuls: the broadcast contracts K=2 over
[s0*ones; s1*ones] x stacked delta-windows of the flat vector, and the
tcol transposes contract K=26 over [s0*I13; s1*I13] x stacked T0/T12
rows, so one NEFF serves all 8 cores with no select pass.

A dummy-matmul burst during the load phase trips the PE HAM clock gate
(1.2 -> 2.4 GHz) before conv1 starts.

Output staging interleaves 4 gram rows per SBUF partition so each 2 MiB
store is one contiguous 16 KiB run per partition (128 descriptors ->
all 16 SDMA engines), alternating between the SP and ACT descriptor
engines. Store phase runs at the HBM write floor (~124us/core when the
paired core's phase is desynced, ~146us when aligned).
"""
import os
import numpy as np

# per-core location starts (each core computes 13 consecutive locations;
# odd cores' 13th overlaps the next core, core 7's 13th is garbage)
_LO = [0, 13, 25, 38, 50, 63, 75, 88]
_CNT = [13, 12, 13, 12, 13, 12, 13, 12]
# conv2 channel-slice starts; delta_k = 1024*lo_k - 100*ch_lo_k is 0 (even k)
# or 12 (odd k)
_CH_LO = [0, 133, 256, 389, 512, 645, 768, 901]
_NSL = 136  # channels per conv2 slice (covers 12 + 13*1024 flat elements)

_CACHE = {}


def _build_nc():
    from concourse import bacc, tile, mybir

    f32 = mybir.dt.float32
    f32r = mybir.dt.float32r
    f16 = mybir.dt.float16
    AF = mybir.ActivationFunctionType

    nc = bacc.Bacc("TRN2", target_bir_lowering=False, debug=False)

    inp_d = nc.dram_tensor("inp", [128, 4, 27, 25], f16, kind="ExternalInput")
    w1_d = nc.dram_tensor("w1t", [128, 4, 9, 512], f16, kind="ExternalInput")
    b1_d = nc.dram_tensor("b1t", [128, 4], f32, kind="ExternalInput")
    w2_d = nc.dram_tensor("w2t", [128, 4, 9, _NSL], f16, kind="ExternalInput")
    b2_d = nc.dram_tensor("b2t", [128, 2], f32, kind="ExternalInput")
    ids2_d = nc.dram_tensor("ids2", [26, 16], f32r, kind="ExternalInput")
    ones2_d = nc.dram_tensor("ones2", [2, 128], f32r, kind="ExternalInput")
    gp_d = nc.dram_tensor("gpart", [13, 1024, 1024], f32, kind="ExternalOutput")
    scr_d = nc.dram_tensor("scratch", [137, 100], f32r)

    with tile.TileContext(nc) as tc:
        with tc.tile_pool(name="consts", bufs=1) as cp, \
             tc.tile_pool(name="work", bufs=1) as wp:

            convp = tc.alloc_tile_pool(name="convp", bufs=1)

            # ---- HAM warm-up: PE clock-gate opens after ~3.4us of
            # sustained activity; burn dummy matmuls while loads stream ----
            dwm = cp.tile([128, 512], f16)
            dw2 = cp.tile([128, 128], f16)
            nc.vector.memset(dwm[:], 0.0)
            nc.vector.memset(dw2[:], 0.0)
            psW = tc.alloc_tile_pool(name="psW", bufs=1, space="PSUM")
            dps = psW.tile([128, 512], f32)
            for _ in range(10):
                nc.tensor.matmul(dps[:], dw2[:], dwm[:], start=True, stop=True)
            psW.release()

            ps1 = tc.alloc_tile_pool(name="ps1", bufs=1, space="PSUM")

            # ---- conv1-critical loads first, alternating HWDGE rings;
            # w1 chunk 0 split by taps so the first matmuls start sooner ----
            insb = convp.tile([128, 4, 27, 25], f16)
            w1sb = convp.tile([128, 4, 9, 512], f16)
            nc.sync.dma_start(out=insb[:, 0], in_=inp_d.ap()[:, 0])
            nc.sync.dma_start(out=w1sb[:, 0, 0:3], in_=w1_d.ap()[:, 0, 0:3])
            nc.scalar.dma_start(out=w1sb[:, 0, 3:9], in_=w1_d.ap()[:, 0, 3:9])
            for c in range(1, 4):
                eng = nc.sync if c % 2 == 0 else nc.scalar
                eng.dma_start(out=insb[:, c], in_=inp_d.ap()[:, c])
                eng.dma_start(out=w1sb[:, c], in_=w1_d.ap()[:, c])

            w2sb = cp.tile([128, 4, 9, _NSL], f16)
            b1sb = cp.tile([128, 4], f32)
            b2sb = cp.tile([128, 2], f32)
            ids2sb = cp.tile([26, 16], f32r)
            ones2sb = cp.tile([2, 128], f32r)

            nc.scalar.dma_start(out=b1sb[:], in_=b1_d.ap())
            nc.scalar.dma_start(out=ids2sb[:], in_=ids2_d.ap())
            nc.scalar.dma_start(out=ones2sb[:], in_=ones2_d.ap())
            nc.scalar.dma_start(out=w2sb[:], in_=w2_d.ap())
            nc.scalar.dma_start(out=b2sb[:], in_=b2_d.ap())

            # ---- conv1: (512,27,25)->(512,23,23), fp16, replicated ----
            c1sb = convp.tile([128, 4, 24, 24], f16)
            nc.vector.memset(c1sb[:, :, 23:24, :], 0.0)
            nc.vector.memset(c1sb[:, :, :, 23:24], 0.0)
            # 8 live accumulation groups, ci-chunk outer so compute overlaps
            # the streaming w1 chunk loads
            c1groups = [(m, r0, nr) for m in range(4)
                        for (r0, nr) in [(0, 12), (12, 11)]]
            c1ps = [ps1.tile([128, 300], f32, tag=f"c1p{gi}",
                             name=f"c1ps{gi}") for gi in range(8)]
            for c in range(4):
                flat_c = insb[:, c].rearrange("p a b -> p (a b)")
                for gi, (m, r0, nr) in enumerate(c1groups):
                    ncols = 25 * nr
                    for t in range(9):
                        dy, dx = t // 3, t % 3
                        s0 = (r0 + dy) * 25 + dx
                        nc.tensor.matmul(
                            c1ps[gi][:, 0:ncols],
                            w1sb[:, c, t, m * 128:(m + 1) * 128],
                            flat_c[:, s0:s0 + ncols],
                            start=(c == 0 and t == 0),
                            stop=(c == 3 and t == 8),
                        )
            for gi, (m, r0, nr) in enumerate(c1groups):
                nc.scalar.activation(
                    out=c1sb[:, m, r0:r0 + nr, 0:23],
                    in_=c1ps[gi][:, 0:300].rearrange("p (a b) -> p a b", b=25)[:, 0:nr, 0:23],
                    func=AF.Relu,
                    bias=b1sb[:, m:m + 1],
                )

            # ---- maxpool 2x2 ceil -> (512,12,12) (pad cells are 0, relu>=0) ----
            colmax = convp.tile([128, 4, 24, 12], f16)
            cpair = c1sb[:].rearrange("p c r (w two) -> p c r w two", two=2)
            nc.vector.tensor_max(colmax[:], cpair[:, :, :, :, 0],
                                 cpair[:, :, :, :, 1])
            pooled = convp.tile([128, 4, 12, 12], f16)
            rpair = colmax[:].rearrange("p c (r two) w -> p c r two w", two=2)
            nc.vector.tensor_max(pooled[:], rpair[:, :, :, 0, :],
                                 rpair[:, :, :, 1, :])

            # ---- conv2 slice: 136 output channels, fp16 ----
            ps1.release()
            ps2 = tc.alloc_tile_pool(name="ps2", bufs=2, space="PSUM")
            c2sb = wp.tile([128, 2, 100], f32)
            for m, (mo, mw) in enumerate([(0, 128), (128, 8)]):
                ps = ps2.tile([128, 100], f32, tag="c2p")
                for c in range(4):
                    for t in range(9):
                        dy, dx = t // 3, t % 3
                        nc.tensor.matmul(
                            ps[0:mw, :],
                            w2sb[:, c, t, mo:mo + mw],
                            pooled[:, c, dy:dy + 10, dx:dx + 10],
                            start=(c == 0 and t == 0),
                            stop=(c == 3 and t == 8),
                        )
                nc.scalar.activation(
                    out=c2sb[0:mw, m, :],
                    in_=ps[0:mw, :],
                    func=AF.Relu,
                    bias=b2sb[0:mw, m:m + 1],
                )
                # flat view via DRAM scratch: store each chunk as it finishes
                if m == 0:
                    nc.sync.dma_start(out=scr_d.ap()[0:128, :],
                                      in_=c2sb[:, 0, :].bitcast(f32r))
                else:
                    nc.sync.dma_start(out=scr_d.ap()[128:136, :],
                                      in_=c2sb[0:8, 1, :].bitcast(f32r))

            # vall2 rows: delta=0 and delta=12 windows of the flat view;
            # T01 stacks the same two windows as 13-row matrices. The s0/s1
            # select is folded into the contraction dim of the gram matmuls.
            flat = scr_d.ap().rearrange("a b -> (a b)")
            vall2 = wp.tile([2, 13324], f32r)
            nc.sync.dma_start(
                out=vall2[0:1], in_=flat[0:13324].rearrange("(p i) -> p i", p=1))
            nc.scalar.dma_start(
                out=vall2[1:2], in_=flat[12:13336].rearrange("(p i) -> p i", p=1))
            T01 = wp.tile([26, 1024], f32r)
            nc.sync.dma_start(
                out=T01[0:13], in_=flat[0:13312].rearrange("(p i) -> p i", i=1024))
            nc.scalar.dma_start(
                out=T01[13:26], in_=flat[12:13324].rearrange("(p i) -> p i", i=1024))

            ps2.release()
            convp.release()

            bp_pool = tc.alloc_tile_pool(name="bcast", bufs=3)
            sp = tc.alloc_tile_pool(name="stage", bufs=6)
            psT = tc.alloc_tile_pool(name="psT", bufs=2, space="PSUM")
            psB = tc.alloc_tile_pool(name="psB", bufs=3, space="PSUM")

            # tcol[p, 4u+x, l] = v_l[512u + 4p + x]  (4-row interleave)
            tcol = wp.tile([128, 8, 16], f32)


            def build_tcol(u):
                l4 = T01[:, 512 * u:512 * (u + 1)].rearrange(
                    "l (m four) -> l four m", four=4)
                for x in range(4):
                    pst = psT.tile([128, 16], f32, tag="tc")
                    nc.tensor.matmul(pst[:, 0:16], l4[:, x, :],
                                     ids2sb[0:26, 0:16],
                                     start=True, stop=True)
                    nc.vector.tensor_copy(tcol[:, 4 * u + x, 0:13],
                                          pst[:, 0:13])

            # ---- Gram outer products, exact fp32 on DVE/ACT ----
            for li in range(13):
                bp = psB.tile([128, 1024], f32, tag="bc")
                for h in range(2):
                    o0 = 1024 * li + 512 * h
                    nc.tensor.matmul(bp[:, 512 * h:512 * h + 512],
                                     ones2sb[0:2, :],
                                     vall2[0:2, o0:o0 + 512],
                                     start=True, stop=True)
                if li == 0:
                    build_tcol(0)
                bc = bp_pool.tile([128, 1024], f32, tag="bcs")
                nc.vector.tensor_copy(bc[:, 0:512], bp[:, 0:512])
                nc.scalar.activation(bc[:, 512:1024], bp[:, 512:1024],
                                     func=AF.Copy)
                for u in range(2):
                    if li == 0 and u == 1:
                        build_tcol(1)
                    st = sp.tile([128, 4096], f32, tag="st")
                    for x in range(4):
                        col = tcol[:, 4 * u + x, li:li + 1]
                        dve = (x % 2 == 0) or (u == 1 and x == 3)
                        if dve:
                            nc.vector.tensor_scalar_mul(
                                st[:, x * 1024:(x + 1) * 1024], bc[:], col)
                        else:
                            nc.scalar.activation(
                                st[:, x * 1024:(x + 1) * 1024], bc[:],
                                func=AF.Copy, scale=col)
                    dst = gp_d.ap()[li, 512 * u:512 * (u + 1), :].rearrange(
                        "(q four) f -> q (four f)", four=4)
                    if u == 0:
                        nc.sync.dma_start(out=dst, in_=st[:])
                    else:
                        nc.scalar.dma_start(out=dst, in_=st[:])
            psB.release()
            psT.release()
            sp.release()
            bp_pool.release()

    nc.compile()
    return nc


def _get_nc():
    if "nc" not in _CACHE:
        _CACHE["nc"] = _build_nc()
    return _CACHE["nc"]


def _host_prep(input, w1, b1, w2, b2):
    x = np.asarray(input, np.float32).reshape(512, 25, 25)
    w1 = np.asarray(w1, np.float32)
    w2 = np.asarray(w2, np.float32)
    b1 = np.asarray(b1, np.float32)
    b2 = np.asarray(b2, np.float32)

    inp = np.zeros((4, 128, 27, 25), np.float32)
    inp[:, :, :25, :] = x.reshape(4, 128, 25, 25)
    inp = np.ascontiguousarray(inp.transpose(1, 0, 2, 3)).astype(np.float16)

    w1t = w1.reshape(512, 512, 9).transpose(1, 2, 0)          # [ci, 9, co]
    w1t = np.ascontiguousarray(
        w1t.reshape(4, 128, 9, 512).transpose(1, 0, 2, 3)).astype(np.float16)
    b1t = np.ascontiguousarray(b1.reshape(4, 128).T)

    common = {"inp": inp, "w1t": w1t, "b1t": b1t}
    in_maps = []
    for k in range(8):
        ch = _CH_LO[k]
        nval = min(1024, ch + _NSL) - ch
        wsl = np.zeros((_NSL, 512, 9), np.float32)
        wsl[:nval] = w2.reshape(1024, 512, 9)[ch:ch + nval]
        w2t = wsl.transpose(1, 2, 0)                           # [512,9,136]
        w2t = np.ascontiguousarray(
            w2t.reshape(4, 128, 9, _NSL).transpose(1, 0, 2, 3)).astype(
                np.float16)
        bsl = np.zeros(256, np.float32)
        bsl[:nval] = b2[ch:ch + nval]
        b2t = np.ascontiguousarray(bsl.reshape(2, 128).T)
        delta_is_12 = (1024 * _LO[k] - 100 * ch) == 12
        s0, s1 = (0.0, 1.0) if delta_is_12 else (1.0, 0.0)
        ids2 = np.zeros((26, 16), np.float32)
        for l in range(13):
            ids2[l, l] = s0
            ids2[13 + l, l] = s1
        ones2 = np.zeros((2, 128), np.float32)
        ones2[0, :] = s0
        ones2[1, :] = s1
        in_maps.append({**common, "w2t": w2t, "b2t": b2t,
                        "ids2": ids2, "ones2": ones2})
    return in_maps


def kernel(input, w1, b1, w2, b2):
    from concourse import bass_utils

    nc = _get_nc()
    in_maps = _host_prep(input, w1, b1, w2, b2)

    prof_dir = os.environ.get("GRAM_KERNEL_PROFILE_DIR")
    if prof_dir:
        from trn_agent_boot.trn_boot import _ntff_profile_via_ctypes
        hook = _ntff_profile_via_ctypes('/opt/axon/libaxon_pjrt.so')
        with hook(prof_dir, [0]):
            res = bass_utils.run_bass_kernel_spmd(
                nc, in_maps, core_ids=list(range(8)))
    else:
        res = bass_utils.run_bass_kernel_spmd(
            nc, in_maps, core_ids=list(range(8)))

    out = np.empty((100, 1024, 1024), np.float32)
    for k in range(8):
        out[_LO[k]:_LO[k] + _CNT[k]] = res.results[k]["gpart"][:_CNT[k]]
    return out
